# revision 24
# baseline (speedup 1.0000x reference)
"""DeepSeek-V3.1 decoder block on 8 Trainium2 NeuronCores (fp8 DoubleRow).

Sharding: core c -> batch b=c//4, position p=c%4; each core owns 4 query
chunks of 128 tokens (chunks {p, 7-p, 8+p, 15-p}) which balances causal
attention. KV projections for the full sequence are replicated within a
batch group (no collectives).

All large GEMMs run as fp8e4m3 DoubleRow matmuls. Dense projections use a
3-term hi/lo split (w_hi*x_hi + w_lo*x_hi + w_hi*x_lo over 256-deep
contraction pairs, 0.75x the bf16 PE cost at ~bf16 accuracy). Attention
scores pack (k_nope | k_rope) into the two DoubleRow slots (one 0.5w
matmul replaces two w-matmuls); PV packs adjacent key tiles; softmax
denominators accumulate on the PE via ones-DoubleRow matmuls. All scales
are powers of two folded into eviction scales / rstat factors, so the
math is exact up to fp8/bf16 rounding.
"""

import sys

sys.path.insert(0, "/opt/trn_rl_repo")

import numpy as np
import ml_dtypes

import concourse.bass as bass
import concourse.mybir as mybir
from concourse import bacc
from concourse.tile import TileContext
from concourse.bass_utils import run_bass_kernel_spmd

B, S, D = 2, 2048, 2048
H, NOPE, ROPE, VH = 16, 128, 64, 128
QL, KVL, FF = 1024, 512, 8192
BASE, EPS = 10000.0, 1e-6
P = 128
T = 512             # query tokens per core
NCH = S // P        # 16 key tiles per batch
NQ = T // P         # 4 q-slots per core
DT = D // P         # 16
DP = DT // 2        # 8 contraction pairs over D
QLT = QL // P       # 8
KVT = KVL // P      # 4
FFT = FF // P       # 64
SM = float(1.0 / np.sqrt(np.float32(NOPE + ROPE)))

# power-of-two fp8 scales
SW = 256.0          # all weights
SX = 8.0            # hidden/x activations
SQ = 8.0            # q_a output
SC = 8.0            # ckv (normalized latent)
SG = 4.0            # gate*up product
SV = 32.0           # v and attn output
SKN = 32.0          # k-side score operands (k_nope, k_rope)
SQN = 256.0         # q-side score operands (carry SM)

F32 = mybir.dt.float32
BF16 = mybir.dt.bfloat16
FP8 = mybir.dt.float8e4
AF = mybir.ActivationFunctionType
ALU = mybir.AluOpType
DR = mybir.MatmulPerfMode.DoubleRow


def chunks_for_pos(p):
    return [p, 7 - p, 8 + p, 15 - p]


# ------------------------------------------------------------------ device

def _rstat(nc, pool, ps_ap, inv_n, scale_sq, out_bcast, chans, width):
    """out_bcast[:chans,:width] = sqrt(scale_sq / (ps*inv_n + eps))."""
    for n in range(width // 512):
        sl = slice(n * 512, (n + 1) * 512)
        ms = pool.tile([1, 512], F32, tag="rs_ms")
        nc.scalar.activation(ms[0:1, :], ps_ap[0:1, sl], AF.Copy, scale=inv_n)
        nc.vector.tensor_scalar_add(ms[0:1, :], ms[0:1, :], EPS)
        inv = pool.tile([1, 512], F32, tag="rs_inv")
        nc.vector.reciprocal(inv[0:1, :], ms[0:1, :])
        r = pool.tile([1, 512], F32, tag="rs_r")
        nc.scalar.activation(r[0:1, :], inv[0:1, :], AF.Sqrt, scale=scale_sq)
        nc.gpsimd.partition_broadcast(out_bcast[0:chans, sl], r[0:1, :],
                                      channels=chans)


def build_program():
    nc = bacc.Bacc("TRN2", target_bir_lowering=False, debug=False, num_devices=8)

    def mm3(out_ap, whi, wlo, xhi, xlo, first, last):
        nc.tensor.matmul(out_ap, whi, xhi, start=first, stop=False,
                         perf_mode=DR)
        nc.tensor.matmul(out_ap, wlo, xhi, start=False, stop=False,
                         perf_mode=DR)
        nc.tensor.matmul(out_ap, whi, xlo, start=False, stop=last,
                         perf_mode=DR)

    # fp8 inputs (host pre-scaled and hi/lo split)
    xT_hi = nc.dram_tensor("xT_hi", [4, P, DT * 512], FP8, kind="ExternalInput")
    xT_lo = nc.dram_tensor("xT_lo", [4, P, DT * 512], FP8, kind="ExternalInput")
    xq_hi = nc.dram_tensor("xq_hi", [P, DT * T], FP8, kind="ExternalInput")
    xq_lo = nc.dram_tensor("xq_lo", [P, DT * T], FP8, kind="ExternalInput")
    xTq32 = nc.dram_tensor("xTq32", [P, DT * T], F32, kind="ExternalInput")
    qaw_hi = nc.dram_tensor("qaw_hi", [2, P, DT * 512], FP8, kind="ExternalInput")
    qaw_lo = nc.dram_tensor("qaw_lo", [2, P, DT * 512], FP8, kind="ExternalInput")
    qbn_hi = nc.dram_tensor("qbn_hi", [H, P, QLT * P], FP8, kind="ExternalInput")
    qbn_lo = nc.dram_tensor("qbn_lo", [H, P, QLT * P], FP8, kind="ExternalInput")
    qbr_hi = nc.dram_tensor("qbr_hi", [H // 2, P, QLT * P], FP8,
                            kind="ExternalInput")
    qbr_lo = nc.dram_tensor("qbr_lo", [H // 2, P, QLT * P], FP8,
                            kind="ExternalInput")
    kva_hi = nc.dram_tensor("kva_hi", [P, DT * (KVL + ROPE)], FP8,
                            kind="ExternalInput")
    kva_lo = nc.dram_tensor("kva_lo", [P, DT * (KVL + ROPE)], FP8,
                            kind="ExternalInput")
    kbk_hi = nc.dram_tensor("kbk_hi", [H, P, KVT * P], FP8, kind="ExternalInput")
    kbk_lo = nc.dram_tensor("kbk_lo", [H, P, KVT * P], FP8, kind="ExternalInput")
    kbv_hi = nc.dram_tensor("kbv_hi", [4, P, KVT * 512], FP8,
                            kind="ExternalInput")
    kbv_lo = nc.dram_tensor("kbv_lo", [4, P, KVT * 512], FP8,
                            kind="ExternalInput")
    ow_hi = nc.dram_tensor("ow_hi", [DT, P, H * P], FP8, kind="ExternalInput")
    ow_lo = nc.dram_tensor("ow_lo", [DT, P, H * P], FP8, kind="ExternalInput")
    gw_hi = nc.dram_tensor("gw_hi", [FFT // 2, P, DT * 256], FP8,
                           kind="ExternalInput")
    gw_lo = nc.dram_tensor("gw_lo", [FFT // 2, P, DT * 256], FP8,
                           kind="ExternalInput")
    uw_hi = nc.dram_tensor("uw_hi", [FFT // 2, P, DT * 256], FP8,
                           kind="ExternalInput")
    uw_lo = nc.dram_tensor("uw_lo", [FFT // 2, P, DT * 256], FP8,
                           kind="ExternalInput")
    dw_hi = nc.dram_tensor("dw_hi", [4, FFT // 8, P, 8 * 512], FP8,
                           kind="ExternalInput")
    dw_lo = nc.dram_tensor("dw_lo", [4, FFT // 8, P, 8 * 512], FP8,
                           kind="ExternalInput")
    cosKT = nc.dram_tensor("cosKT", [ROPE, S], BF16, kind="ExternalInput")
    sinKT = nc.dram_tensor("sinKT", [ROPE, S], BF16, kind="ExternalInput")
    cosQ2 = nc.dram_tensor("cosQ2", [P, T], F32, kind="ExternalInput")
    sinQ2 = nc.dram_tensor("sinQ2", [P, T], F32, kind="ExternalInput")
    p2t = nc.dram_tensor("p2t", [P, P], BF16, kind="ExternalInput")
    maskM = nc.dram_tensor("maskM", [P, NCH * P], BF16, kind="ExternalInput")
    ones_in = nc.dram_tensor("ones_in", [P, 1], BF16, kind="ExternalInput")
    ones8_in = nc.dram_tensor("ones8_in", [P, 32], FP8, kind="ExternalInput")
    out = nc.dram_tensor("out", [D, T], F32, kind="ExternalOutput")

    with TileContext(nc) as tc:
        with tc.tile_pool(name="pp", bufs=1) as pp, \
             tc.tile_pool(name="dram", bufs=1, space="DRAM") as dp:

            # ---------------- persistent constants & cross-stage tensors
            ones_sb = pp.tile([P, 1], BF16, tag="ones")
            ones8_sb = pp.tile([P, 32], FP8, tag="ones8")
            nc.sync.dma_start(ones_sb[:], ones_in[:, :])
            nc.sync.dma_start(ones8_sb[:], ones8_in[:, :])

            p2t_sb = pp.tile([P, P], BF16, tag="p2t")
            mask_sb = pp.tile([P, NCH * P], BF16, tag="mask")
            at_hi = pp.tile([P, H * T], FP8, tag="at_hi")
            at_lo = pp.tile([P, H * T], FP8, tag="at_lo")

            kfin8_dram = dp.tile([P, S], FP8, tag="kfin8")
            khT8_dram = dp.tile([H, P, 4, 512], FP8, tag="khT8")

            vpool_cm = tc.tile_pool(name="vp", bufs=1)
            vp = vpool_cm.__enter__()
            v8_sb = vp.tile([P, NCH * H * VH], FP8, tag="v8")
            qj8 = vp.tile([P, 2 * H * T], FP8, tag="qj8")
            # zero the rope slot's pad rows once (killed rows for the joint
            # score matmul; rows 0:64 are overwritten per head)
            nc.vector.memset(qj8[64:128, H * T:2 * H * T], 0.0)

            # ============================ stage A: kv path over full sequence
            with tc.tile_pool(name="a_pp", bufs=1) as app, \
                 tc.tile_pool(name="a_st", bufs=2) as ast, \
                 tc.tile_pool(name="a_sc", bufs=3) as asc, \
                 tc.tile_pool(name="a_sq", bufs=1) as asq:

                kwh = app.tile([P, DT * (KVL + ROPE)], FP8, tag="kwh")
                kwl = app.tile([P, DT * (KVL + ROPE)], FP8, tag="kwl")
                ckv_hi = app.tile([P, KVT * S], FP8, tag="ckv_hi")
                ckv_lo = app.tile([P, KVT * S], FP8, tag="ckv_lo")
                rx_b = app.tile([ROPE, S], F32, tag="rx_b")
                kr_bf = app.tile([ROPE, S], BF16, tag="krbf")
                kfin8_sb = app.tile([ROPE, S], FP8, tag="kfin8s")
                cosK_sb = app.tile([ROPE, S], BF16, tag="cosK")
                sinK_sb = app.tile([ROPE, S], BF16, tag="sinK")

                a1b = tc.tile_pool(name="a_p1", bufs=2, space="PSUM")
                ap1 = a1b.__enter__()
                a1 = tc.tile_pool(name="a_ps", bufs=1, space="PSUM")
                aps = a1.__enter__()

                KW = KVL + ROPE
                kwh_r = kwh[:].rearrange("p (d c) -> p d c", d=DT)
                kwl_r = kwl[:].rearrange("p (d c) -> p d c", d=DT)

                for n in range(S // 512):
                    pts = [aps.tile([P, 512], F32, name=f"kva{m}", tag=f"kva{m}")
                           for m in range(5)]
                    psx = ap1.tile([1, 512], F32, tag="ps1")
                    xth = ast.tile([P, DT * 512], FP8, tag="xth", bufs=2)
                    xtl = ast.tile([P, DT * 512], FP8, tag="xtl", bufs=2)
                    if n == 0:
                        # fine-grained startup: first compute unit's inputs
                        # arrive first (kw pair dp, then xt pair dp)
                        for dpi in range(DP):
                            nc.sync.dma_start(
                                kwh[:, dpi * 2 * KW:(dpi + 1) * 2 * KW],
                                kva_hi[:, dpi * 2 * KW:(dpi + 1) * 2 * KW])
                            nc.sync.dma_start(
                                xth[:, dpi * 1024:(dpi + 1) * 1024],
                                xT_hi[n, :, dpi * 1024:(dpi + 1) * 1024])
                            nc.sync.dma_start(
                                kwl[:, dpi * 2 * KW:(dpi + 1) * 2 * KW],
                                kva_lo[:, dpi * 2 * KW:(dpi + 1) * 2 * KW])
                            nc.sync.dma_start(
                                xtl[:, dpi * 1024:(dpi + 1) * 1024],
                                xT_lo[n, :, dpi * 1024:(dpi + 1) * 1024])
                            if dpi == 1:
                                nc.sync.dma_start(cosK_sb[:], cosKT[:, :])
                            if dpi == 3:
                                nc.sync.dma_start(sinK_sb[:], sinKT[:, :])
                    else:
                        nc.sync.dma_start(xth[:], xT_hi[n, :, :])
                        nc.sync.dma_start(xtl[:], xT_lo[n, :, :])
                    if n == 1:
                        nc.sync.dma_start(p2t_sb[:], p2t[:, :])
                        nc.sync.dma_start(mask_sb[:], maskM[:, :])

                    xth_r = xth[:].rearrange("p (d c) -> p d c", d=DT)
                    xtl_r = xtl[:].rearrange("p (d c) -> p d c", d=DT)
                    xacc = asc.tile([P, 512], BF16, tag="xacc", bufs=2)
                    for dt in range(DT):
                        xt = xth[:, dt * 512:(dt + 1) * 512]
                        if dt == 0:
                            nc.vector.tensor_tensor(xacc[:], xt, xt, ALU.mult)
                        else:
                            sq = asc.tile([P, 512], BF16, tag="sqx", bufs=4)
                            eng = nc.gpsimd if dt % 2 == 0 else nc.vector
                            eng.tensor_tensor(sq[:], xt, xt, ALU.mult)
                            nc.vector.tensor_tensor(xacc[:], xacc[:], sq[:],
                                                    ALU.add)
                    for dpi in range(DP):
                        xh = xth_r[:, 2 * dpi:2 * dpi + 2, :]
                        xl = xtl_r[:, 2 * dpi:2 * dpi + 2, :]
                        first = dpi == 0
                        last = dpi == DP - 1
                        for m in range(5):
                            mp = P if m < 4 else ROPE
                            c0, c1 = m * P, m * P + mp
                            mm3(pts[m][:mp, :],
                                kwh_r[:, 2 * dpi:2 * dpi + 2, c0:c1],
                                kwl_r[:, 2 * dpi:2 * dpi + 2, c0:c1],
                                xh, xl, first, last)
                    nc.tensor.matmul(psx[0:1, :], ones_sb[:], xacc[:],
                                     start=True, stop=True)
                    _rstat(nc, asq, psx, 1.0 / (D * SX * SX),
                           (SKN / (SW * SX)) ** 2,
                           rx_b[:, n * 512:(n + 1) * 512], ROPE, 512)
                    # ckv chunk: evict, stats, normalize, hi/lo split inline
                    ckv_cn = asc.tile([P, KVT * 512], BF16, tag="ckvn",
                                      bufs=2)
                    for m in range(4):
                        nc.scalar.activation(ckv_cn[:, m * 512:(m + 1) * 512],
                                             pts[m][:], AF.Copy,
                                             scale=SC / (SW * SX))
                    nc.scalar.activation(kr_bf[:, n * 512:(n + 1) * 512],
                                         pts[4][0:ROPE, :], AF.Copy)
                    pskv = ap1.tile([1, 512], F32, tag="ps1")
                    for kvt in range(KVT):
                        sq2 = asc.tile([P, 512], BF16, tag="sq")
                        nc.gpsimd.tensor_tensor(
                            sq2[:], ckv_cn[:, kvt * 512:(kvt + 1) * 512],
                            ckv_cn[:, kvt * 512:(kvt + 1) * 512], ALU.mult)
                        nc.tensor.matmul(pskv[0:1, :], ones_sb[:], sq2[:],
                                         start=(kvt == 0), stop=(kvt == KVT - 1))
                    rkv_n = asc.tile([P, 512], F32, tag="rkvn", bufs=2)
                    _rstat(nc, asq, pskv, 1.0 / (KVL * SC * SC), 1.0,
                           rkv_n[:], P, 512)
                    for kvt in range(KVT):
                        cs = slice(kvt * 512, (kvt + 1) * 512)
                        sl = slice(kvt * S + n * 512, kvt * S + n * 512 + 512)
                        nc.vector.tensor_tensor(ckv_cn[:, cs], ckv_cn[:, cs],
                                                rkv_n[:], ALU.mult)
                        nc.vector.tensor_copy(ckv_hi[:, sl], ckv_cn[:, cs])
                        nc.vector.tensor_tensor(ckv_lo[:, sl], ckv_cn[:, cs],
                                                ckv_hi[:, sl], ALU.subtract)

                a1.__exit__(None, None, None)
                a1b.__exit__(None, None, None)

                # k_rope rot + cos/sin + rx scale -> kfin8
                a3 = tc.tile_pool(name="a_p3", bufs=2, space="PSUM")
                ap3 = a3.__enter__()
                for n in range(S // 512):
                    pr = ap3.tile([P, 512], F32, tag="rot")
                    nc.tensor.matmul(pr[0:ROPE, :], p2t_sb[0:ROPE, 0:ROPE],
                                     kr_bf[:, n * 512:(n + 1) * 512],
                                     start=True, stop=True)
                    m1t = asc.tile([ROPE, 512], F32, tag="km1", bufs=2)
                    nc.vector.tensor_tensor(m1t[:],
                                            kr_bf[:, n * 512:(n + 1) * 512],
                                            cosK_sb[:, n * 512:(n + 1) * 512],
                                            ALU.mult)
                    t2 = asc.tile([ROPE, 512], F32, tag="kt2", bufs=2)
                    nc.vector.tensor_tensor(t2[:], pr[0:ROPE, :],
                                            sinK_sb[:, n * 512:(n + 1) * 512],
                                            ALU.mult)
                    nc.vector.tensor_tensor(t2[:], t2[:], m1t[:], ALU.add)
                    nc.vector.tensor_tensor(kfin8_sb[:, n * 512:(n + 1) * 512],
                                            t2[:],
                                            rx_b[:, n * 512:(n + 1) * 512],
                                            ALU.mult)
                nc.sync.dma_start(kfin8_dram[0:ROPE, :], kfin8_sb[:])
                z64 = asc.tile([ROPE, S], FP8, tag="z64", bufs=1)
                nc.vector.memset(z64[:], 0.0)
                nc.sync.dma_start(kfin8_dram[ROPE:P, :], z64[:])

                ckvh_r = ckv_hi[:].rearrange("p (k s) -> p k s", k=KVT)
                ckvl_r = ckv_lo[:].rearrange("p (k s) -> p k s", k=KVT)

                # kv_b k-half: khs8[h] -> DRAM
                for h in range(H):
                    kbkh = ast.tile([P, KVT * P], FP8, tag="kbkh")
                    kbkl = ast.tile([P, KVT * P], FP8, tag="kbkl")
                    nc.sync.dma_start(kbkh[:], kbk_hi[h, :, :])
                    nc.sync.dma_start(kbkl[:], kbk_lo[h, :, :])
                    kbkh_r = kbkh[:].rearrange("p (k c) -> p k c", k=KVT)
                    kbkl_r = kbkl[:].rearrange("p (k c) -> p k c", k=KVT)
                    for n in range(S // 512):
                        pt = ap3.tile([P, 512], F32, tag="kb", bufs=3)
                        for kp in range(KVT // 2):
                            mm3(pt[:],
                                kbkh_r[:, 2 * kp:2 * kp + 2, :],
                                kbkl_r[:, 2 * kp:2 * kp + 2, :],
                                ckvh_r[:, 2 * kp:2 * kp + 2,
                                       n * 512:(n + 1) * 512],
                                ckvl_r[:, 2 * kp:2 * kp + 2,
                                       n * 512:(n + 1) * 512],
                                kp == 0, kp == KVT // 2 - 1)
                        khs = asc.tile([P, 512], FP8, tag="khs")
                        nc.scalar.activation(khs[:], pt[:], AF.Copy,
                                             scale=SKN / (SW * SC))
                        nc.sync.dma_start(khT8_dram[h, :, n, :], khs[:])

                # kv_b v-half: v8 rows=token, cols=(h,vh) -> SBUF resident
                for n4 in range(4):
                    kbvh = ast.tile([P, KVT * 512], FP8, tag="kbvh", bufs=2)
                    kbvl = ast.tile([P, KVT * 512], FP8, tag="kbvl", bufs=2)
                    nc.sync.dma_start(kbvh[:], kbv_hi[n4, :, :])
                    nc.sync.dma_start(kbvl[:], kbv_lo[n4, :, :])
                    kbvh_r = kbvh[:].rearrange("p (k c) -> p k c", k=KVT)
                    kbvl_r = kbvl[:].rearrange("p (k c) -> p k c", k=KVT)
                    for tt in range(NCH):
                        pt = ap3.tile([P, 512], F32, tag="vb", bufs=3)
                        for kp in range(KVT // 2):
                            mm3(pt[:],
                                ckvh_r[:, 2 * kp:2 * kp + 2, tt * P:(tt + 1) * P],
                                ckvl_r[:, 2 * kp:2 * kp + 2, tt * P:(tt + 1) * P],
                                kbvh_r[:, 2 * kp:2 * kp + 2, :],
                                kbvl_r[:, 2 * kp:2 * kp + 2, :],
                                kp == 0, kp == KVT // 2 - 1)
                        nc.scalar.activation(
                            v8_sb[:, tt * H * VH + n4 * 512:
                                  tt * H * VH + (n4 + 1) * 512],
                            pt[:], AF.Copy, scale=SV / (SW * SC))
                a3.__exit__(None, None, None)

            # ============================ stage B-pre: q_a (core's T tokens)
            bq_cm = tc.tile_pool(name="bq_pp", bufs=1)
            bqp = bq_cm.__enter__()
            qa_c = bqp.tile([P, QLT * T], BF16, tag="qa_c")
            qa_hi = bqp.tile([P, QLT * T], FP8, tag="qa_hi")
            qa_lo = bqp.tile([P, QLT * T], FP8, tag="qa_lo")
            rq_b = bqp.tile([P, T], F32, tag="rq_b")
            cosQ_sb = bqp.tile([P, T], F32, tag="cosQ")
            sinQ_sb = bqp.tile([P, T], F32, tag="sinQ")
            xqh_all = bqp.tile([P, DT * T], FP8, tag="xqh")
            xql_all = bqp.tile([P, DT * T], FP8, tag="xql")
            nc.sync.dma_start(xqh_all[:], xq_hi[:, :])

            with tc.tile_pool(name="b_st", bufs=1) as bst, \
                 tc.tile_pool(name="b_sc", bufs=3) as bsc:
                xqh_r = xqh_all[:].rearrange("p (d c) -> p d c", d=DT)
                xql_r = xql_all[:].rearrange("p (d c) -> p d c", d=DT)

                b1 = tc.tile_pool(name="b_ps", bufs=1, space="PSUM")
                bps = b1.__enter__()
                psq = bps.tile([1, T], F32, tag="psq")
                for half in range(2):
                    pts = [bps.tile([P, T], F32, name=f"qa{m}", tag=f"qa{m}")
                           for m in range(4)]
                    qwh = bst.tile([P, DT * 512], FP8, tag="qwh")
                    qwl = bst.tile([P, DT * 512], FP8, tag="qwl")
                    nc.sync.dma_start(qwh[:], qaw_hi[half, :, :])
                    if half == 0:
                        nc.sync.dma_start(xql_all[:], xq_lo[:, :])
                    nc.sync.dma_start(qwl[:], qaw_lo[half, :, :])
                    qwh_r = qwh[:].rearrange("p (d c) -> p d c", d=DT)
                    qwl_r = qwl[:].rearrange("p (d c) -> p d c", d=DT)
                    if half == 0:
                        nc.sync.dma_start(cosQ_sb[:], cosQ2[:, :])
                        nc.sync.dma_start(sinQ_sb[:], sinQ2[:, :])
                    for ti, (wr, xr) in enumerate(
                            [(qwh_r, xqh_r), (qwl_r, xqh_r), (qwh_r, xql_r)]):
                        for dpi in range(DP):
                            for m in range(4):
                                c0, c1 = m * P, (m + 1) * P
                                nc.tensor.matmul(
                                    pts[m][:],
                                    wr[:, 2 * dpi:2 * dpi + 2, c0:c1],
                                    xr[:, 2 * dpi:2 * dpi + 2, :],
                                    start=(ti == 0 and dpi == 0),
                                    stop=(ti == 2 and dpi == DP - 1),
                                    perf_mode=DR)
                    for m in range(4):
                        mi = half * 4 + m
                        sl = slice(mi * T, (mi + 1) * T)
                        nc.scalar.activation(qa_c[:, sl], pts[m][:], AF.Copy,
                                             scale=SQ / (SW * SX))
                        nc.vector.tensor_copy(qa_hi[:, sl], qa_c[:, sl])
                        nc.vector.tensor_tensor(qa_lo[:, sl], qa_c[:, sl],
                                                qa_hi[:, sl], ALU.subtract)
                        sqb = bsc.tile([P, T], BF16, tag="sqb")
                        nc.vector.tensor_tensor(sqb[:], qa_c[:, sl],
                                                qa_c[:, sl], ALU.mult)
                        nc.tensor.matmul(psq[0:1, :], ones_sb[:], sqb[:],
                                         start=(mi == 0), stop=(mi == QLT - 1))
                _rstat(nc, bqp, psq, 1.0 / (QL * SQ * SQ),
                       (SM * SQN / (SW * SQ)) ** 2, rq_b, P, T)
                b1.__exit__(None, None, None)

            # ============================ fused stage C: q_b + attention
            qah_r = qa_hi[:].rearrange("p (k c) -> p k c", k=QLT)
            qal_r = qa_lo[:].rearrange("p (k c) -> p k c", k=QLT)
            qj_r = qj8[:].rearrange("p (t x) -> p t x", t=2)
            v8_r = v8_sb[:].rearrange("p (k c) -> p k c", k=NCH)
            ones8_r = ones8_sb[:].rearrange("p (t o) -> p t o", t=2)[:, :, 0:1]

            with tc.tile_pool(name="c_st", bufs=3) as cst, \
                 tc.tile_pool(name="c_w", bufs=2) as cw, \
                 tc.tile_pool(name="c_pr", bufs=3) as cpr, \
                 tc.tile_pool(name="c_ps", bufs=1, space="PSUM") as cps, \
                 tc.tile_pool(name="c_pt", bufs=2, space="PSUM") as cpt, \
                 tc.tile_pool(name="c_pq", bufs=2, space="PSUM") as cpq:

                def emit_qb(h):
                    """q_b nope (and rope when h is even) for head h -> qj8."""
                    nbh = cw.tile([P, QLT * P], FP8, tag="nbh")
                    nbl = cw.tile([P, QLT * P], FP8, tag="nbl")
                    nc.sync.dma_start(nbh[:], qbn_hi[h, :, :])
                    nc.sync.dma_start(nbl[:], qbn_lo[h, :, :])
                    nbh_r = nbh[:].rearrange("p (k c) -> p k c", k=QLT)
                    nbl_r = nbl[:].rearrange("p (k c) -> p k c", k=QLT)
                    pt = cpq.tile([P, T], F32, tag="qb")
                    for mp in range(QLT // 2):
                        mm3(pt[:],
                            nbh_r[:, 2 * mp:2 * mp + 2, :],
                            nbl_r[:, 2 * mp:2 * mp + 2, :],
                            qah_r[:, 2 * mp:2 * mp + 2, :],
                            qal_r[:, 2 * mp:2 * mp + 2, :],
                            mp == 0, mp == QLT // 2 - 1)
                    nc.vector.tensor_tensor(qj8[:, h * T:(h + 1) * T], pt[:],
                                            rq_b[:], ALU.mult)
                    if h % 2 == 0:
                        g = h // 2
                        rbh = cw.tile([P, QLT * P], FP8, tag="rbh")
                        rbl = cw.tile([P, QLT * P], FP8, tag="rbl")
                        nc.sync.dma_start(rbh[:], qbr_hi[g, :, :])
                        nc.sync.dma_start(rbl[:], qbr_lo[g, :, :])
                        rbh_r = rbh[:].rearrange("p (k c) -> p k c", k=QLT)
                        rbl_r = rbl[:].rearrange("p (k c) -> p k c", k=QLT)
                        ptr = cpq.tile([P, T], F32, tag="qb", name="qbr")
                        for mp in range(QLT // 2):
                            mm3(ptr[:],
                                rbh_r[:, 2 * mp:2 * mp + 2, :],
                                rbl_r[:, 2 * mp:2 * mp + 2, :],
                                qah_r[:, 2 * mp:2 * mp + 2, :],
                                qal_r[:, 2 * mp:2 * mp + 2, :],
                                mp == 0, mp == QLT // 2 - 1)
                        qrb = cpr.tile([P, T], BF16, tag="qrb")
                        nc.scalar.activation(qrb[:], ptr[:], AF.Copy)
                        mm1 = cpr.tile([P, T], F32, tag="mm1")
                        nc.vector.tensor_tensor(mm1[:], ptr[:], cosQ_sb[:],
                                                ALU.mult)
                        prot = cpq.tile([P, T], F32, tag="qb", name="rot")
                        nc.tensor.matmul(prot[:], p2t_sb[:], qrb[:],
                                         start=True, stop=True)
                        t2 = cpr.tile([P, T], F32, tag="bt2")
                        nc.vector.tensor_tensor(t2[:], prot[:], sinQ_sb[:],
                                                ALU.mult)
                        nc.vector.tensor_tensor(t2[:], t2[:], mm1[:], ALU.add)
                        qrf = cpr.tile([P, T], FP8, tag="qrf")
                        nc.vector.tensor_tensor(qrf[:], t2[:], rq_b[:],
                                                ALU.mult)
                        base = H * T
                        nc.sync.dma_start(
                            qj8[0:ROPE, base + 2 * g * T:base + (2 * g + 1) * T],
                            qrf[0:ROPE, :])
                        nc.sync.dma_start(
                            qj8[0:ROPE,
                                base + (2 * g + 1) * T:base + (2 * g + 2) * T],
                            qrf[ROPE:P, :])

                emit_qb(0)
                emit_qb(1)
                for h in range(H):
                    kj = cst.tile([P, 2 * S], FP8, tag="kj")
                    nc.sync.dma_start(kj[:, 0:S], khT8_dram[h, :, :, :])
                    nc.sync.dma_start(kj[:, S:2 * S], kfin8_dram[:, :])
                    kj_r = kj[:].rearrange("p (t s) -> p t s", t=2)
                    pa = cps.tile([P, T], F32, tag="pa")
                    ps = cps.tile([1, T], F32, tag="ps")

                    for m in range(NCH // 2):
                        j0 = m // 2
                        q0 = j0 * P
                        w = T - q0
                        pt2 = cpt.tile([P, 2 * T], F32, tag="pt2")
                        pt2_r = pt2[:].rearrange("p (t x) -> p t x", t=2)
                        for t in range(2):
                            kt = 2 * m + t
                            nc.tensor.matmul(
                                pt2[:, t * T:t * T + w],
                                kj_r[:, :, kt * P:(kt + 1) * P],
                                qj_r[:, :, h * T + q0:h * T + T],
                                start=True, stop=True, perf_mode=DR)
                        probs2 = cpr.tile([P, 2 * T], FP8, tag="probs2",
                                          name="probs2")
                        probs2_r = probs2[:].rearrange("p (t x) -> p t x", t=2)
                        nc.scalar.activation(probs2_r[:, :, 0:w],
                                             pt2_r[:, :, 0:w], AF.Exp,
                                             scale=1.0 / (SKN * SQN))
                        for t in range(2):
                            kt = 2 * m + t
                            eng = nc.vector if t == 0 else nc.gpsimd
                            eng.tensor_tensor(
                                probs2[:, t * T:t * T + P],
                                probs2[:, t * T:t * T + P],
                                mask_sb[:, kt * P:(kt + 1) * P], ALU.mult)
                        nc.tensor.matmul(ps[0:1, q0:T], ones8_r,
                                         probs2_r[:, :, 0:w],
                                         start=(m == 0), stop=(m == NCH // 2 - 1),
                                         perf_mode=DR)
                        for j in range(j0, NQ):
                            c0 = j * P - q0
                            nc.tensor.matmul(
                                pa[:, j * P:(j + 1) * P],
                                v8_r[:, 2 * m:2 * m + 2, h * VH:(h + 1) * VH],
                                probs2_r[:, :, c0:c0 + P],
                                start=(m == 0 and j == 0),
                                stop=(m == 2 * j + 1), perf_mode=DR)

                    if h + 2 < H:
                        emit_qb(h + 2)
                    rs = cpr.tile([1, T], F32, tag="rs")
                    nc.vector.reciprocal(rs[0:1, :], ps[0:1, :])
                    rsb = cpr.tile([P, T], F32, tag="rsb")
                    nc.gpsimd.partition_broadcast(rsb[:], rs[0:1, :], channels=P)
                    atc = cpr.tile([P, T], BF16, tag="atc")
                    nc.vector.tensor_tensor(atc[:], pa[:], rsb[:], ALU.mult)
                    nc.vector.tensor_copy(at_hi[:, h * T:(h + 1) * T], atc[:])
                    nc.vector.tensor_tensor(at_lo[:, h * T:(h + 1) * T],
                                            atc[:], at_hi[:, h * T:(h + 1) * T],
                                            ALU.subtract)

            bq_cm.__exit__(None, None, None)
            vpool_cm.__exit__(None, None, None)

            # ===== persistents for D..F (opened after the A..C pools close)
            fpp_cm = tc.tile_pool(name="f_pp", bufs=1)
            fpp = fpp_cm.__enter__()
            x2T = fpp.tile([P, DT * T], F32, tag="x2T")
            x2hi = fpp.tile([P, DT * T], FP8, tag="x2hi")
            x2lo = fpp.tile([P, DT * T], FP8, tag="x2lo")
            gu_hi = fpp.tile([P, FFT * T], FP8, tag="gu_hi")
            gu_lo = fpp.tile([P, FFT * T], FP8, tag="gu_lo")

            # ============================ stage D: o_proj + residual + norm
            ath_r = at_hi[:].rearrange("p (k c) -> p k c", k=H)
            atl_r = at_lo[:].rearrange("p (k c) -> p k c", k=H)

            est_cm = tc.tile_pool(name="e_st", bufs=2)
            est = est_cm.__enter__()
            epre = {}
            with tc.tile_pool(name="d_st", bufs=1) as dst, \
                 tc.tile_pool(name="d_w", bufs=2) as dw, \
                 tc.tile_pool(name="d_sc", bufs=3) as dsc, \
                 tc.tile_pool(name="d_ps", bufs=3, space="PSUM") as dps:
                # prefetch stage E's first gate/up block
                for nm, src in [("gwh", gw_hi), ("gwl", gw_lo),
                                ("uwh", uw_hi), ("uwl", uw_lo)]:
                    tile = est.tile([P, DT * 256], FP8, tag=nm, name=nm + "0")
                    nc.sync.dma_start(tile[:], src[0, :, :])
                    epre[nm] = tile

                dp1 = tc.tile_pool(name="d_p1", bufs=1, space="PSUM")
                dps1 = dp1.__enter__()
                ps2 = dps1.tile([1, T], F32, tag="ps2")
                for dt in range(DT):
                    odh = dw.tile([P, H * P], FP8, tag="odh")
                    odl = dw.tile([P, H * P], FP8, tag="odl")
                    nc.sync.dma_start(odh[:], ow_hi[dt, :, :])
                    nc.sync.dma_start(odl[:], ow_lo[dt, :, :])
                    odh_r = odh[:].rearrange("p (k c) -> p k c", k=H)
                    odl_r = odl[:].rearrange("p (k c) -> p k c", k=H)
                    po = dps.tile([P, T], F32, tag="po")
                    for hp in range(H // 2):
                        mm3(po[:],
                            odh_r[:, 2 * hp:2 * hp + 2, :],
                            odl_r[:, 2 * hp:2 * hp + 2, :],
                            ath_r[:, 2 * hp:2 * hp + 2, :],
                            atl_r[:, 2 * hp:2 * hp + 2, :],
                            hp == 0, hp == H // 2 - 1)
                    poc = dsc.tile([P, T], F32, tag="poc")
                    nc.scalar.activation(poc[:], po[:], AF.Copy,
                                         scale=1.0 / (SW * SV))
                    xq32_t = dsc.tile([P, T], F32, tag="xq32")
                    nc.sync.dma_start(xq32_t[:], xTq32[:, dt * T:(dt + 1) * T])
                    nc.vector.tensor_tensor(x2T[:, dt * T:(dt + 1) * T], poc[:],
                                            xq32_t[:], ALU.add)
                    sqd = dsc.tile([P, T], BF16, tag="sqd")
                    nc.gpsimd.tensor_tensor(sqd[:], x2T[:, dt * T:(dt + 1) * T],
                                            x2T[:, dt * T:(dt + 1) * T],
                                            ALU.mult)
                    nc.tensor.matmul(ps2[0:1, :], ones_sb[:], sqd[:],
                                     start=(dt == 0), stop=(dt == DT - 1))
                r2b = dst.tile([P, T], F32, tag="r2b")
                _rstat(nc, dst, ps2, 1.0 / D, SX * SX, r2b, P, T)
                dp1.__exit__(None, None, None)
                for dt in range(DT):
                    sl = slice(dt * T, (dt + 1) * T)
                    x2c = dsc.tile([P, T], BF16, tag="x2c")
                    nc.vector.tensor_tensor(x2c[:], x2T[:, sl], r2b[:],
                                            ALU.mult)
                    nc.scalar.activation(x2hi[:, sl], x2c[:], AF.Copy)
                    nc.vector.tensor_tensor(x2lo[:, sl], x2c[:], x2hi[:, sl],
                                            ALU.subtract)

            # ============================ stage E: FFN gate/up -> gu hi/lo
            x2h_r = x2hi[:].rearrange("p (d c) -> p d c", d=DT)
            x2l_r = x2lo[:].rearrange("p (d c) -> p d c", d=DT)

            fst_cm = tc.tile_pool(name="f_st", bufs=2)
            fst = fst_cm.__enter__()
            pre = {}
            with tc.tile_pool(name="e_sc", bufs=3) as esc, \
                 tc.tile_pool(name="e_ps", bufs=3, space="PSUM") as eps:
                # prefetch stage F's first down_w blocks
                for k in range(2):
                    dwbh = fst.tile([P, 8 * 512], FP8, tag=f"dwh{k}",
                                    name=f"dwh{k}")
                    dwbl = fst.tile([P, 8 * 512], FP8, tag=f"dwl{k}",
                                    name=f"dwl{k}")
                    nc.sync.dma_start(dwbh[:], dw_hi[k, 0, :, :])
                    nc.sync.dma_start(dwbl[:], dw_lo[k, 0, :, :])
                    pre[f"dwh{k}"] = dwbh
                    pre[f"dwl{k}"] = dwbl
                for fb in range(FFT // 2):
                    if fb == 0:
                        gwh, gwl = epre["gwh"], epre["gwl"]
                        uwh, uwl = epre["uwh"], epre["uwl"]
                    else:
                        gwh = est.tile([P, DT * 256], FP8, tag="gwh")
                        gwl = est.tile([P, DT * 256], FP8, tag="gwl")
                        uwh = est.tile([P, DT * 256], FP8, tag="uwh")
                        uwl = est.tile([P, DT * 256], FP8, tag="uwl")
                        nc.sync.dma_start(gwh[:], gw_hi[fb, :, :])
                        nc.sync.dma_start(gwl[:], gw_lo[fb, :, :])
                        nc.sync.dma_start(uwh[:], uw_hi[fb, :, :])
                        nc.sync.dma_start(uwl[:], uw_lo[fb, :, :])
                    gwh_r = gwh[:].rearrange("p (d c) -> p d c", d=DT)
                    gwl_r = gwl[:].rearrange("p (d c) -> p d c", d=DT)
                    uwh_r = uwh[:].rearrange("p (d c) -> p d c", d=DT)
                    uwl_r = uwl[:].rearrange("p (d c) -> p d c", d=DT)
                    for i in range(2):
                        f = fb * 2 + i
                        pg = eps.tile([P, T], F32, tag="pg")
                        pu = eps.tile([P, T], F32, tag="pu")
                        for dpi in range(DP):
                            c0, c1 = i * P, (i + 1) * P
                            mm3(pg[:],
                                gwh_r[:, 2 * dpi:2 * dpi + 2, c0:c1],
                                gwl_r[:, 2 * dpi:2 * dpi + 2, c0:c1],
                                x2h_r[:, 2 * dpi:2 * dpi + 2, :],
                                x2l_r[:, 2 * dpi:2 * dpi + 2, :],
                                dpi == 0, dpi == DP - 1)
                        for dpi in range(DP):
                            c0, c1 = i * P, (i + 1) * P
                            mm3(pu[:],
                                uwh_r[:, 2 * dpi:2 * dpi + 2, c0:c1],
                                uwl_r[:, 2 * dpi:2 * dpi + 2, c0:c1],
                                x2h_r[:, 2 * dpi:2 * dpi + 2, :],
                                x2l_r[:, 2 * dpi:2 * dpi + 2, :],
                                dpi == 0, dpi == DP - 1)
                        gs = esc.tile([P, T], BF16, tag="gs")
                        nc.scalar.activation(gs[:], pg[:], AF.Silu,
                                             scale=1.0 / (SW * SX))
                        gus = esc.tile([P, T], BF16, tag="gus")
                        nc.vector.tensor_tensor(gus[:], gs[:], pu[:], ALU.mult)
                        guc = esc.tile([P, T], BF16, tag="guc")
                        nc.scalar.activation(guc[:], gus[:], AF.Copy,
                                             scale=SG / (SW * SX))
                        sl = slice(f * T, (f + 1) * T)
                        nc.vector.tensor_copy(gu_hi[:, sl], guc[:])
                        nc.vector.tensor_tensor(gu_lo[:, sl], guc[:],
                                                gu_hi[:, sl], ALU.subtract)

            # ============================ stage F: down proj + residual
            guh_r = gu_hi[:].rearrange("p (k c) -> p k c", k=FFT)
            gul_r = gu_lo[:].rearrange("p (k c) -> p k c", k=FFT)
            with tc.tile_pool(name="f_sc", bufs=2) as fsc, \
                 tc.tile_pool(name="f_ps", bufs=1, space="PSUM") as fps:
                for np2 in range(D // 1024):
                    pds = [fps.tile([P, T], F32, name=f"pd{i}", tag=f"pd{i}")
                           for i in range(2 * NQ)]
                    for fb8 in range(FFT // 8):
                        if np2 == 0 and fb8 == 0:
                            dwbhs = [pre["dwh0"], pre["dwh1"]]
                            dwbls = [pre["dwl0"], pre["dwl1"]]
                        else:
                            dwbhs, dwbls = [], []
                            for k in range(2):
                                dwbh = fst.tile([P, 8 * 512], FP8,
                                                tag=f"dwh{k}", name=f"dwh{k}")
                                dwbl = fst.tile([P, 8 * 512], FP8,
                                                tag=f"dwl{k}", name=f"dwl{k}")
                                nc.sync.dma_start(
                                    dwbh[:], dw_hi[np2 * 2 + k, fb8, :, :])
                                nc.sync.dma_start(
                                    dwbl[:], dw_lo[np2 * 2 + k, fb8, :, :])
                                dwbhs.append(dwbh)
                                dwbls.append(dwbl)
                        dwh_rs = [d[:].rearrange("p (f c) -> p f c", f=8)
                                  for d in dwbhs]
                        dwl_rs = [d[:].rearrange("p (f c) -> p f c", f=8)
                                  for d in dwbls]
                        last_blk = np2 == 1 and fb8 == FFT // 8 - 1
                        # bank-major order in the final block staggers the
                        # stop/evict of the 8 banks so the tail is one bank
                        if last_blk:
                            order = [(fip, k, i)
                                     for k in range(2) for i in range(4)
                                     for fip in range(4)]
                        else:
                            order = [(fip, k, i)
                                     for fip in range(4) for k in range(2)
                                     for i in range(4)]
                        for fip, k, i in order:
                            fpair = fb8 * 8 + 2 * fip
                            first = fb8 == 0 and fip == 0
                            last = fb8 == FFT // 8 - 1 and fip == 3
                            mm3(pds[k * NQ + i][:],
                                dwh_rs[k][:, 2 * fip:2 * fip + 2,
                                          i * P:(i + 1) * P],
                                dwl_rs[k][:, 2 * fip:2 * fip + 2,
                                          i * P:(i + 1) * P],
                                guh_r[:, fpair:fpair + 2, :],
                                gul_r[:, fpair:fpair + 2, :],
                                first, last)
                        if last_blk:
                            for k in range(2):
                                for i in range(4):
                                    dt = (np2 * 2 + k) * 4 + i
                                    _evict_out(nc, fsc, pds[k * NQ + i], x2T,
                                               out, dt, alt=(i % 2 == 1))
                    if np2 != 1:
                        for k in range(2):
                            for i in range(4):
                                dt = (np2 * 2 + k) * 4 + i
                                _evict_out(nc, fsc, pds[k * NQ + i], x2T,
                                           out, dt)

            fst_cm.__exit__(None, None, None)
            est_cm.__exit__(None, None, None)
            fpp_cm.__exit__(None, None, None)

    nc.compile()
    return nc


def _evict_out(nc, pool, pd, x2T, out, dt, alt=False):
    pdc = pool.tile([P, T], F32, tag="pdc")
    if alt:
        nc.vector.tensor_scalar_mul(pdc[:], pd[:], 1.0 / (SW * SG))
    else:
        nc.scalar.activation(pdc[:], pd[:], AF.Copy, scale=1.0 / (SW * SG))
    ot = pool.tile([P, T], F32, tag="ot")
    eng = nc.gpsimd if alt else nc.vector
    eng.tensor_tensor(ot[:], pdc[:], x2T[:, dt * T:(dt + 1) * T], ALU.add)
    nc.sync.dma_start(out[dt * P:(dt + 1) * P, :], ot[:])
